# revision 18
# baseline (speedup 1.0000x reference)
"""Causal self-attention on 8 TRN2 NeuronCores.

Problem: x[2,2048,1024], wq/wk/wv/wo[1024,1024] (nn.Linear convention,
out = y @ W.T), H=16 heads, D=64, causal softmax, f32 in/out.

Sharding: tensor-parallel over heads x data-parallel over batch.
Core i handles batch b=i//4 and head group g=i%4 (4 heads each).
wq/wk/wv are split row-wise (output-feature) per head group; wo is
split column-wise; each core returns a partial output projection
out_partial[b] and the host sums the 4 partials per batch.

v2 design notes (HW-trace driven):
- Everything the PE touches is bf16 (inputs converted on host): enables
  fast weight load (fp32-HIGH matmuls disable FWL) and halves HBM
  traffic.  PSUM accumulation stays f32; exp runs f32-in/bf16-out.
- The TensorE instruction stream is kept dense so the HAM clock gate
  stays at K=8/8 (2.4 GHz): per head the scores matmuls for key-chunk
  ki+2 are issued BEFORE the PV matmuls of ki, so ScalarE's exp of one
  chunk overlaps the next chunk's matmuls instead of stalling the PE
  queue.  The exp stream is the per-head rate limiter, so independent
  "filler" matmul work is interleaved into each head's gaps: the V
  projection during head 0, the m=1 half of the K/Q projections during
  head 1, and the output projection (spans 0-2) during head 3.
- Softmax row sums ride a ones-column folded into V (PV stationary is
  [128 keys, 4*65] with a ones lane per head); normalization takes the
  reciprocal of the PSUM sum row on one DVE lane and broadcasts it to
  64 partitions via a small DRAM bounce with clean 2 KiB strides (the
  old 128-lane transpose bounce generated thousands of 16 B software-DGE
  descriptors).
- PSUM budget: 2 x mg[128,1024]f32 (scores / filler, 4 banks) +
  4 x pv[128,512]f32 (one per 512-query span, 4 banks) = all 8 banks.
  The lead-in K/Q m=0 projection time-shares the pv banks (generation 0)
  before any PV accumulation starts.
"""

import sys

for _p in ("/opt/trn_rl_repo", "/root/.axon_site"):
    if _p not in sys.path:
        sys.path.insert(0, _p)

import numpy as np
from ml_dtypes import bfloat16

import concourse.bass as bass
import concourse.mybir as mybir
import concourse.tile as tile
from concourse import bacc
from concourse.bass_utils import run_bass_kernel_spmd

B, T, C, H = 2, 2048, 1024, 16
DH = C // H            # 64 head dim
HG = 4                 # heads per core
GW = HG * DH           # 256 features per head group
NB = T // 128          # 16 key chunks
NS = T // 512          # 4 query spans
KC = C // 128          # 8 contraction chunks over C
SCALE = 1.0 / float(np.sqrt(DH))
N_CORES = 8

F32 = mybir.dt.float32
BF16 = mybir.dt.bfloat16
EXP = mybir.ActivationFunctionType.Exp
COPY = mybir.ActivationFunctionType.Copy


def build_nc():
    nc = bacc.Bacc("TRN2", target_bir_lowering=False, debug=False,
                   num_devices=N_CORES)
    xT = nc.declare_dram_parameter("xT", [C, T], BF16, isOutput=False)
    wqT = nc.declare_dram_parameter("wqT", [128, KC * GW], BF16,
                                    isOutput=False)
    wkT = nc.declare_dram_parameter("wkT", [128, KC * GW], BF16,
                                    isOutput=False)
    wvT = nc.declare_dram_parameter("wvT", [128, KC * GW], BF16,
                                    isOutput=False)
    woT = nc.declare_dram_parameter("woT", [GW, C], BF16, isOutput=False)
    outT = nc.declare_dram_parameter("outT", [C, T], F32, isOutput=True)
    r_dram = nc.dram_tensor("r_scratch", [HG, NS, 512], F32)
    rr_dram = nc.dram_tensor("rr_scratch", [HG, NS, 512], F32)

    with tile.TileContext(nc) as tc:
        with tc.tile_pool(name="pers", bufs=1) as pers:
            # ---- persistent SBUF tensors; DMAs in consumption order ----
            # merged weight tiles: one [128, KC*GW] tile per matrix, one
            # DMA start each (chunk i at cols [i*GW:(i+1)*GW]); x loads are
            # whole chunks -- fewer, larger descriptors (issue-rate bound)
            wq_all = pers.tile([128, KC * GW], BF16, tag="wqa", name="wqa")
            wk_all = pers.tile([128, KC * GW], BF16, tag="wka", name="wka")
            wv_all = pers.tile([128, KC * GW], BF16, tag="wva", name="wva")
            wq_t = [wq_all[:, i * GW:(i + 1) * GW] for i in range(KC)]
            wk_t = [wk_all[:, i * GW:(i + 1) * GW] for i in range(KC)]
            wv_t = [wv_all[:, i * GW:(i + 1) * GW] for i in range(KC)]
            # weights arrive host-pre-interleaved as the exact SBUF
            # image ([128, KC*GW]): one 128x4KB-descriptor DMA each
            nc.scalar.dma_start(out=wq_all, in_=wqT[:, :])
            nc.scalar.dma_start(out=wk_all, in_=wkT[:, :])
            xts = [pers.tile([128, T], BF16, tag=f"xT{i}", name=f"xT{i}")
                   for i in range(KC)]
            for i in range(KC):
                nc.sync.dma_start(out=xts[i],
                                  in_=xT[i * 128:(i + 1) * 128, :])
            wo_t = [pers.tile([128, C], BF16, tag=f"wo{j}", name=f"wo{j}")
                    for j in range(2)]
            nc.scalar.dma_start(out=wv_all, in_=wvT[:, :])
            for j in range(2):
                nc.scalar.dma_start(
                    out=wo_t[j], in_=woT[j * 128:(j + 1) * 128, :])

            qts = [pers.tile([128, T], BF16, tag=f"qT{m}", name=f"qT{m}")
                   for m in range(2)]
            kts = [pers.tile([128, T], BF16, tag=f"kT{m}", name=f"kT{m}")
                   for m in range(2)]
            yts = [pers.tile([128, T], BF16, tag=f"yT{m}", name=f"yT{m}")
                   for m in range(2)]
            vts = [pers.tile([128, HG * 65], BF16, tag=f"V{tb}",
                             name=f"V{tb}") for tb in range(NB)]

            # bf16 triangular mask for the diagonal 128x128 strip of
            # P^T: keep (1) where col >= row i.e. q >= k, else 0
            trim = pers.tile([128, 128], BF16, tag="trim", name="trim")
            nc.gpsimd.memset(trim, 1.0)
            nc.gpsimd.affine_select(
                out=trim, in_=trim, compare_op=mybir.AluOpType.is_ge,
                fill=0.0, base=0, pattern=[[1, 128]], channel_multiplier=-1)
            ones4 = pers.tile([128, 4], BF16, tag="ones4", name="ones4")
            for j in range(4):
                nc.scalar.activation(
                    out=ones4[:, j:j + 1],
                    in_=nc.const_aps.tensor(1.0, [128, 1]), func=COPY)

            with tc.tile_pool(name="mgs", bufs=2, space="PSUM") as mgs, \
                 tc.tile_pool(name="pvs", bufs=1, space="PSUM") as pvs, \
                 tc.tile_pool(name="ptp", bufs=14) as ptp, \
                 tc.tile_pool(name="rp", bufs=2) as rp, \
                 tc.tile_pool(name="ost", bufs=4) as ost:

                # prefetch the exp table set during the lead-in
                warm = rp.tile([128, 1], BF16, tag="warm", name="warm")
                nc.scalar.activation(
                    out=warm, in_=nc.const_aps.tensor(0.0, [128, 1]),
                    func=EXP)

                # ---- lead-in: Q m=0 (all spans) + K m=0 span 0 through
                # the pv banks (generation 0) ----
                def lead_proj(wt, dest, s, tag):
                    ps = pvs.tile([128, 512], F32, tag=tag, name=tag)
                    for k in range(KC):
                        nc.tensor.matmul(
                            ps, wt[k][:, 0:128],
                            xts[k][:, s * 512:(s + 1) * 512],
                            start=(k == 0), stop=(k == KC - 1))
                    nc.vector.tensor_copy(
                        out=dest[0][:, s * 512:(s + 1) * 512], in_=ps)

                for s in range(NS):
                    lead_proj(wq_t, qts, s, f"pv{s}")
                lead_proj(wk_t, kts, 0, "pv0")

                # ---- filler emitters (each borrows one mg generation) ----
                def v_chunk(tb):
                    mg = mgs.tile([128, 1024], F32, tag="mg", name="mg")
                    for k in range(KC):
                        nc.tensor.matmul(
                            mg[:, 0:GW],
                            xts[k][:, tb * 128:(tb + 1) * 128], wv_t[k],
                            start=(k == 0), stop=(k == KC - 1))
                    vt = vts[tb]
                    nc.vector.tensor_copy(
                        out=vt.rearrange("p (h c) -> p h c", c=65)[:, :, 0:64],
                        in_=mg[:, 0:GW].rearrange("p (h c) -> p h c", c=64))
                    nc.vector.tensor_copy(
                        out=vt.rearrange("p (h c) -> p h c", c=65)[:, :, 64],
                        in_=ones4)

                def kq_chunk(wt, dest, m, sp):
                    # one 512-wide span of the m projection in one mg tile
                    mg = mgs.tile([128, 1024], F32, tag="mg", name="mg")
                    for k in range(KC):
                        nc.tensor.matmul(
                            mg[:, 0:512],
                            wt[k][:, m * 128:(m + 1) * 128],
                            xts[k][:, sp * 512:(sp + 1) * 512],
                            start=(k == 0), stop=(k == KC - 1))
                    nc.vector.tensor_copy(
                        out=dest[m][:, sp * 512:(sp + 1) * 512],
                        in_=mg[:, 0:512])

                def out_pairs(pairs):
                    # up to two (m, s) output-projection tiles per mg gen
                    mg = mgs.tile([128, 1024], F32, tag="mg", name="mg")
                    for idx, (m, s) in enumerate(pairs):
                        for jj in range(2):
                            nc.tensor.matmul(
                                mg[:, idx * 512:(idx + 1) * 512],
                                wo_t[jj][:, m * 128:(m + 1) * 128],
                                yts[jj][:, s * 512:(s + 1) * 512],
                                start=(jj == 0), stop=(jj == 1))
                        ot = ost.tile([128, 512], F32, tag="ot", name="ot")
                        nc.vector.tensor_copy(
                            out=ot, in_=mg[:, idx * 512:(idx + 1) * 512])
                        nc.gpsimd.dma_start(
                            out=outT[m * 128:(m + 1) * 128,
                                     s * 512:(s + 1) * 512],
                            in_=ot)

                # ---- attention emitters ----
                def emit_scores(h, ki, dup=0):
                    qt, kt = qts[h // 2], kts[h // 2]
                    po = (h % 2) * 64
                    smin, j = ki // 4, ki % 4
                    nsp = NS - smin
                    c0 = 128 * j
                    halves = []
                    for hb in range(0, nsp, 2):
                        nh = min(2, nsp - hb)
                        off = c0 if hb == 0 else 0
                        w = nh * 512
                        mg = mgs.tile([128, 1024], F32, tag="mg", name="mg")
                        for t2 in range(hb, hb + nh):
                            s = smin + t2
                            lo = (t2 - hb) * 512 + (c0 if t2 == 0 else 0)
                            # reps > 1 re-issues the same matmul: an
                            # idempotent overwrite that keeps the PE
                            # activity monitor from re-throttling the
                            # clock through exp-bound stretches
                            reps = 1 + (dup if t2 == hb else 0)
                            for _ in range(reps):
                                nc.tensor.matmul(
                                    mg[:, lo:(t2 - hb + 1) * 512],
                                    kt[po:po + 64, ki * 128:(ki + 1) * 128],
                                    qt[po:po + 64,
                                       s * 512 + (c0 if t2 == 0 else 0):
                                       (s + 1) * 512],
                                    start=True, stop=True)
                        pt = ptp.tile([128, 1024], BF16, tag="pt", name="pt")
                        nc.scalar.activation(
                            out=pt[:, off:w], in_=mg[:, off:w],
                            func=EXP, scale=SCALE)
                        halves.append(pt)
                    nc.gpsimd.tensor_mul(
                        out=halves[0][:, c0:c0 + 128],
                        in0=halves[0][:, c0:c0 + 128], in1=trim)
                    return halves

                def emit_pv(h, ki, halves, pv):
                    smin, j = ki // 4, ki % 4
                    nsp = NS - smin
                    c0 = 128 * j
                    done = []
                    for t2 in range(nsp):
                        s = smin + t2
                        pt = halves[t2 // 2]
                        lo = (t2 % 2) * 512 + (c0 if t2 == 0 else 0)
                        nc.tensor.matmul(
                            pv[s][0:65, (c0 if t2 == 0 else 0):512],
                            vts[ki][:, h * 65:(h + 1) * 65],
                            pt[:, lo:(t2 % 2 + 1) * 512],
                            start=(ki == 0), stop=(ki == 4 * s + 3))
                        if ki == 4 * s + 3:
                            done.append(s)
                    return done

                def finalize(h, s, pvt):
                    # 1/rowsum with the 512 sums respread over 4 partitions
                    # (DVE iterative-op cost scales with free size, so
                    # [4,128] is ~4x cheaper than [1,512]); two small DRAM
                    # bounces with clean >=512B descriptors
                    po = (h % 2) * 64
                    srow = rp.tile([1, 512], F32, tag="srow", name="srow")
                    nc.vector.tensor_copy(out=srow, in_=pvt[64:65, :])
                    nc.sync.dma_start(out=r_dram[h, s, :], in_=srow)
                    st4 = rp.tile([4, 128], F32, tag="st4", name="st4")
                    nc.sync.dma_start(
                        out=st4,
                        in_=r_dram[h, s, :].rearrange("(p c) -> p c", p=4))
                    rc4 = rp.tile([4, 128], F32, tag="rc4", name="rc4")
                    nc.vector.reciprocal(out=rc4, in_=st4)
                    nc.sync.dma_start(
                        out=rr_dram[h, s, :].rearrange("(p c) -> p c", p=4),
                        in_=rc4)
                    rb = rp.tile([64, 512], F32, tag="rb", name="rb")
                    rsl = rr_dram[h, s, :]
                    nc.sync.dma_start(
                        out=rb,
                        in_=bass.AP(tensor=rsl.tensor, offset=rsl.offset,
                                    ap=[[0, 64]] + list(rsl.ap)))
                    nc.vector.tensor_mul(
                        out=yts[h // 2][po:po + 64, s * 512:(s + 1) * 512],
                        in0=pvt[0:64, :], in1=rb)

                # ---- main loop: one global slot per (head, key-chunk),
                # scores emitted 4 slots ahead of their PV (so the next
                # head's scores preroll during this head's last slots) ----
                # filler deques; deadlines: V(tb) before PV(h0, tb);
                # K m=0 span sp before scores(h0, 4*sp); K/Q m=1 before
                # scores(h2, *) i.e. by global slot ~28.
                fills = {
                    -1: [lambda: v_chunk(0),
                         lambda: kq_chunk(wk_t, kts, 0, 1),
                         lambda: v_chunk(1),
                         lambda: kq_chunk(wk_t, kts, 0, 2),
                         lambda: v_chunk(2),
                         lambda: v_chunk(3),
                         lambda: v_chunk(4)],
                    0: [lambda: kq_chunk(wk_t, kts, 0, 3)]
                       + [lambda tb=t: v_chunk(tb) for t in range(5, 16)],
                    1: [lambda sp=sp: kq_chunk(wq_t, qts, 1, sp)
                        for sp in range(NS)]
                       + [lambda: kq_chunk(wk_t, kts, 1, 0),
                          lambda: kq_chunk(wk_t, kts, 1, 1)],
                    2: [lambda sp=sp: kq_chunk(wk_t, kts, 1, sp)
                        for sp in (2, 3)],
                    3: [],
                }

                GT = HG * NB
                live = {}
                out_work = []
                pv = None
                for g in range(-7, GT):
                    if g + 7 < GT:
                        hg = g // NB if g >= 0 else -1
                        dup = (g >= 0 and not fills[hg]
                               and not (hg == 3 and len(out_work) >= 2))
                        live[g + 7] = emit_scores(*divmod(g + 7, NB),
                                                  dup=1 if dup else 0)
                    if g < 0:
                        if fills[-1]:
                            fills[-1].pop(0)()
                        continue
                    h, ki = divmod(g, NB)
                    if ki == 0:
                        pv = [pvs.tile([128, 512], F32, tag=f"pv{s}",
                                       name=f"pv{s}") for s in range(NS)]
                    if fills[h]:
                        fills[h].pop(0)()
                    elif h == 3 and len(out_work) >= 2:
                        out_pairs([out_work.pop(0), out_work.pop(0)])
                    done = emit_pv(h, ki, live.pop(g), pv)
                    for s in done:
                        finalize(h, s, pv[s])
                        if h == 3:
                            out_work.extend((m, s) for m in range(8))
                # tail: remaining output-projection tiles
                while out_work:
                    out_pairs([out_work.pop(0)
                               for _ in range(min(2, len(out_work)))])
    nc.compile()
    return nc


_NC_CACHE = None


def _get_nc():
    global _NC_CACHE
    if _NC_CACHE is None:
        _NC_CACHE = build_nc()
    return _NC_CACHE


def _w_img(w):
    # SBUF image of a [GW, C] weight slice: wT chunk i ([128, GW]) at
    # columns [i*GW:(i+1)*GW] of the [128, KC*GW] tile
    wT = np.ascontiguousarray(w.T)                     # [C, GW]
    img = wT.reshape(KC, 128, GW).transpose(1, 0, 2).reshape(128, KC * GW)
    return np.ascontiguousarray(img).astype(bfloat16)


def make_in_maps(x, wq, wk, wv, wo):
    x = np.asarray(x, dtype=np.float32)
    wq = np.asarray(wq, dtype=np.float32)
    wk = np.asarray(wk, dtype=np.float32)
    wv = np.asarray(wv, dtype=np.float32)
    wo = np.asarray(wo, dtype=np.float32)
    in_maps = []
    for core in range(N_CORES):
        b, g = core // HG, core % HG
        rows = slice(g * GW, (g + 1) * GW)
        in_maps.append({
            "xT": np.ascontiguousarray(x[b].T).astype(bfloat16),
            "wqT": _w_img(wq[rows, :]),
            "wkT": _w_img(wk[rows, :]),
            "wvT": _w_img(wv[rows, :]),
            "woT": np.ascontiguousarray(wo[:, rows].T).astype(bfloat16),
        })
    return in_maps


def run(x, wq, wk, wv, wo, trace=False, tmpdir=None):
    nc = _get_nc()
    in_maps = make_in_maps(x, wq, wk, wv, wo)
    res = run_bass_kernel_spmd(nc, in_maps, core_ids=list(range(N_CORES)),
                               trace=trace, tmpdir=tmpdir)
    out = np.zeros((B, T, C), dtype=np.float32)
    for core in range(N_CORES):
        out[core // HG] += res.results[core]["outT"].T
    return out, res


def kernel(x, wq, wk, wv, wo):
    out, _ = run(x, wq, wk, wv, wo)
    return out


# revision 19
# speedup vs baseline: 1.2627x; 1.2627x over previous
"""Causal self-attention on 8 TRN2 NeuronCores.

Problem: x[2,2048,1024], wq/wk/wv/wo[1024,1024] (nn.Linear convention,
out = y @ W.T), H=16 heads, D=64, causal softmax, f32 in/out.

Sharding: tensor-parallel over heads x data-parallel over batch.
Core i handles batch b=i//4 and head group g=i%4 (4 heads each).
wq/wk/wv are split row-wise (output-feature) per head group; wo is
split column-wise; each core returns a partial output projection
out_partial[b] and the host sums the 4 partials per batch.

v2 design notes (HW-trace driven):
- Everything the PE touches is bf16 (inputs converted on host): enables
  fast weight load (fp32-HIGH matmuls disable FWL) and halves HBM
  traffic.  PSUM accumulation stays f32; exp runs f32-in/bf16-out.
- The TensorE instruction stream is kept dense so the HAM clock gate
  stays at K=8/8 (2.4 GHz): per head the scores matmuls for key-chunk
  ki+2 are issued BEFORE the PV matmuls of ki, so ScalarE's exp of one
  chunk overlaps the next chunk's matmuls instead of stalling the PE
  queue.  The exp stream is the per-head rate limiter, so independent
  "filler" matmul work is interleaved into each head's gaps: the V
  projection during head 0, the m=1 half of the K/Q projections during
  head 1, and the output projection (spans 0-2) during head 3.
- Softmax row sums ride a ones-column folded into V (PV stationary is
  [128 keys, 4*65] with a ones lane per head); normalization takes the
  reciprocal of the PSUM sum row on one DVE lane and broadcasts it to
  64 partitions via a small DRAM bounce with clean 2 KiB strides (the
  old 128-lane transpose bounce generated thousands of 16 B software-DGE
  descriptors).
- PSUM budget: 2 x mg[128,1024]f32 (scores / filler, 4 banks) +
  4 x pv[128,512]f32 (one per 512-query span, 4 banks) = all 8 banks.
  The lead-in K/Q m=0 projection time-shares the pv banks (generation 0)
  before any PV accumulation starts.
"""

import sys

for _p in ("/opt/trn_rl_repo", "/root/.axon_site"):
    if _p not in sys.path:
        sys.path.insert(0, _p)

import numpy as np
from ml_dtypes import bfloat16

import concourse.bass as bass
import concourse.mybir as mybir
import concourse.tile as tile
from concourse import bacc
from concourse.bass_utils import run_bass_kernel_spmd

B, T, C, H = 2, 2048, 1024, 16
DH = C // H            # 64 head dim
HG = 4                 # heads per core
GW = HG * DH           # 256 features per head group
NB = T // 128          # 16 key chunks
NS = T // 512          # 4 query spans
KC = C // 128          # 8 contraction chunks over C
SCALE = 1.0 / float(np.sqrt(DH))
N_CORES = 8

F32 = mybir.dt.float32
BF16 = mybir.dt.bfloat16
EXP = mybir.ActivationFunctionType.Exp
COPY = mybir.ActivationFunctionType.Copy


def build_nc():
    nc = bacc.Bacc("TRN2", target_bir_lowering=False, debug=False,
                   num_devices=N_CORES)
    xT = nc.declare_dram_parameter("xT", [C, T], BF16, isOutput=False)
    wqT = nc.declare_dram_parameter("wqT", [128, KC * GW], BF16,
                                    isOutput=False)
    wkT = nc.declare_dram_parameter("wkT", [128, KC * GW], BF16,
                                    isOutput=False)
    wvT = nc.declare_dram_parameter("wvT", [128, KC * GW], BF16,
                                    isOutput=False)
    woT = nc.declare_dram_parameter("woT", [GW, C], BF16, isOutput=False)
    outT = nc.declare_dram_parameter("outT", [C, T], F32, isOutput=True)
    r_dram = nc.dram_tensor("r_scratch", [HG, NS, 512], F32)
    rr_dram = nc.dram_tensor("rr_scratch", [HG, NS, 512], F32)

    with tile.TileContext(nc) as tc:
        with tc.tile_pool(name="pers", bufs=1) as pers:
            # ---- persistent SBUF tensors; DMAs in consumption order ----
            # merged weight tiles: one [128, KC*GW] tile per matrix, one
            # DMA start each (chunk i at cols [i*GW:(i+1)*GW]); x loads are
            # whole chunks -- fewer, larger descriptors (issue-rate bound)
            wq_all = pers.tile([128, KC * GW], BF16, tag="wqa", name="wqa")
            wk_all = pers.tile([128, KC * GW], BF16, tag="wka", name="wka")
            wv_all = pers.tile([128, KC * GW], BF16, tag="wva", name="wva")
            wq_t = [wq_all[:, i * GW:(i + 1) * GW] for i in range(KC)]
            wk_t = [wk_all[:, i * GW:(i + 1) * GW] for i in range(KC)]
            wv_t = [wv_all[:, i * GW:(i + 1) * GW] for i in range(KC)]
            # weights arrive host-pre-interleaved as the exact SBUF
            # image ([128, KC*GW]): one 128x4KB-descriptor DMA each
            nc.scalar.dma_start(out=wq_all, in_=wqT[:, :])
            nc.scalar.dma_start(out=wk_all, in_=wkT[:, :])
            xts = [pers.tile([128, T], BF16, tag=f"xT{i}", name=f"xT{i}")
                   for i in range(KC)]
            for i in range(KC):
                nc.sync.dma_start(out=xts[i],
                                  in_=xT[i * 128:(i + 1) * 128, :])
            wo_t = [pers.tile([128, C], BF16, tag=f"wo{j}", name=f"wo{j}")
                    for j in range(2)]
            nc.scalar.dma_start(out=wv_all, in_=wvT[:, :])
            for j in range(2):
                nc.scalar.dma_start(
                    out=wo_t[j], in_=woT[j * 128:(j + 1) * 128, :])

            qts = [pers.tile([128, T], BF16, tag=f"qT{m}", name=f"qT{m}")
                   for m in range(2)]
            kts = [pers.tile([128, T], BF16, tag=f"kT{m}", name=f"kT{m}")
                   for m in range(2)]
            yts = [pers.tile([128, T], BF16, tag=f"yT{m}", name=f"yT{m}")
                   for m in range(2)]
            vts = [pers.tile([128, HG * 65], BF16, tag=f"V{tb}",
                             name=f"V{tb}") for tb in range(NB)]

            # bf16 triangular mask for the diagonal 128x128 strip of
            # P^T: keep (1) where col >= row i.e. q >= k, else 0
            trim = pers.tile([128, 128], BF16, tag="trim", name="trim")
            nc.gpsimd.memset(trim, 1.0)
            nc.gpsimd.affine_select(
                out=trim, in_=trim, compare_op=mybir.AluOpType.is_ge,
                fill=0.0, base=0, pattern=[[1, 128]], channel_multiplier=-1)
            ones4 = pers.tile([128, 4], BF16, tag="ones4", name="ones4")
            for j in range(4):
                nc.scalar.activation(
                    out=ones4[:, j:j + 1],
                    in_=nc.const_aps.tensor(1.0, [128, 1]), func=COPY)

            with tc.tile_pool(name="mgs", bufs=2, space="PSUM") as mgs, \
                 tc.tile_pool(name="pvs", bufs=1, space="PSUM") as pvs, \
                 tc.tile_pool(name="ptp", bufs=14) as ptp, \
                 tc.tile_pool(name="rp", bufs=2) as rp, \
                 tc.tile_pool(name="ost", bufs=4) as ost:

                # prefetch the exp table set during the lead-in
                warm = rp.tile([128, 1], BF16, tag="warm", name="warm")
                nc.scalar.activation(
                    out=warm, in_=nc.const_aps.tensor(0.0, [128, 1]),
                    func=EXP)

                # ---- lead-in: Q m=0 (all spans) + K m=0 span 0 through
                # the pv banks (generation 0) ----
                def lead_proj(wt, dest, s, tag):
                    ps = pvs.tile([128, 512], F32, tag=tag, name=tag)
                    for k in range(KC):
                        nc.tensor.matmul(
                            ps, wt[k][:, 0:128],
                            xts[k][:, s * 512:(s + 1) * 512],
                            start=(k == 0), stop=(k == KC - 1))
                    nc.vector.tensor_copy(
                        out=dest[0][:, s * 512:(s + 1) * 512], in_=ps)

                for s in range(NS):
                    lead_proj(wq_t, qts, s, f"pv{s}")
                lead_proj(wk_t, kts, 0, "pv0")

                # ---- filler emitters (each borrows one mg generation) ----
                def v_chunk(tb):
                    mg = mgs.tile([128, 1024], F32, tag="mg", name="mg")
                    for k in range(KC):
                        nc.tensor.matmul(
                            mg[:, 0:GW],
                            xts[k][:, tb * 128:(tb + 1) * 128], wv_t[k],
                            start=(k == 0), stop=(k == KC - 1))
                    vt = vts[tb]
                    nc.vector.tensor_copy(
                        out=vt.rearrange("p (h c) -> p h c", c=65)[:, :, 0:64],
                        in_=mg[:, 0:GW].rearrange("p (h c) -> p h c", c=64))
                    nc.vector.tensor_copy(
                        out=vt.rearrange("p (h c) -> p h c", c=65)[:, :, 64],
                        in_=ones4)

                def kq_chunk(wt, dest, m, sp):
                    # one 512-wide span of the m projection in one mg tile
                    mg = mgs.tile([128, 1024], F32, tag="mg", name="mg")
                    for k in range(KC):
                        nc.tensor.matmul(
                            mg[:, 0:512],
                            wt[k][:, m * 128:(m + 1) * 128],
                            xts[k][:, sp * 512:(sp + 1) * 512],
                            start=(k == 0), stop=(k == KC - 1))
                    nc.vector.tensor_copy(
                        out=dest[m][:, sp * 512:(sp + 1) * 512],
                        in_=mg[:, 0:512])

                def out_pairs(pairs):
                    # up to two (m, s) output-projection tiles per mg gen
                    mg = mgs.tile([128, 1024], F32, tag="mg", name="mg")
                    for idx, (m, s) in enumerate(pairs):
                        for jj in range(2):
                            nc.tensor.matmul(
                                mg[:, idx * 512:(idx + 1) * 512],
                                wo_t[jj][:, m * 128:(m + 1) * 128],
                                yts[jj][:, s * 512:(s + 1) * 512],
                                start=(jj == 0), stop=(jj == 1))
                        ot = ost.tile([128, 512], F32, tag="ot", name="ot")
                        nc.vector.tensor_copy(
                            out=ot, in_=mg[:, idx * 512:(idx + 1) * 512])
                        nc.gpsimd.dma_start(
                            out=outT[m * 128:(m + 1) * 128,
                                     s * 512:(s + 1) * 512],
                            in_=ot)

                # ---- attention emitters ----
                def emit_scores(h, ki, dup=0):
                    qt, kt = qts[h // 2], kts[h // 2]
                    po = (h % 2) * 64
                    smin, j = ki // 4, ki % 4
                    nsp = NS - smin
                    c0 = 128 * j
                    halves = []
                    for hb in range(0, nsp, 2):
                        nh = min(2, nsp - hb)
                        off = c0 if hb == 0 else 0
                        w = nh * 512
                        mg = mgs.tile([128, 1024], F32, tag="mg", name="mg")
                        for t2 in range(hb, hb + nh):
                            s = smin + t2
                            lo = (t2 - hb) * 512 + (c0 if t2 == 0 else 0)
                            # reps > 1 re-issues the same matmul: an
                            # idempotent overwrite that keeps the PE
                            # activity monitor from re-throttling the
                            # clock through exp-bound stretches
                            reps = 1 + (dup if t2 == hb else 0)
                            for _ in range(reps):
                                nc.tensor.matmul(
                                    mg[:, lo:(t2 - hb + 1) * 512],
                                    kt[po:po + 64, ki * 128:(ki + 1) * 128],
                                    qt[po:po + 64,
                                       s * 512 + (c0 if t2 == 0 else 0):
                                       (s + 1) * 512],
                                    start=True, stop=True)
                        pt = ptp.tile([128, 1024], BF16, tag="pt", name="pt")
                        nc.scalar.activation(
                            out=pt[:, off:w], in_=mg[:, off:w],
                            func=EXP, scale=SCALE)
                        halves.append(pt)
                    nc.gpsimd.tensor_mul(
                        out=halves[0][:, c0:c0 + 128],
                        in0=halves[0][:, c0:c0 + 128], in1=trim)
                    return halves

                def emit_pv(h, ki, halves, pv):
                    smin, j = ki // 4, ki % 4
                    nsp = NS - smin
                    c0 = 128 * j
                    done = []
                    for t2 in range(nsp):
                        s = smin + t2
                        pt = halves[t2 // 2]
                        lo = (t2 % 2) * 512 + (c0 if t2 == 0 else 0)
                        nc.tensor.matmul(
                            pv[s][0:65, (c0 if t2 == 0 else 0):512],
                            vts[ki][:, h * 65:(h + 1) * 65],
                            pt[:, lo:(t2 % 2 + 1) * 512],
                            start=(ki == 0), stop=(ki == 4 * s + 3))
                        if ki == 4 * s + 3:
                            done.append(s)
                    return done

                def finalize(h, s, pvt):
                    # 1/rowsum with the 512 sums respread over 4 partitions
                    # (DVE iterative-op cost scales with free size, so
                    # [4,128] is ~4x cheaper than [1,512]); two small DRAM
                    # bounces with clean >=512B descriptors
                    po = (h % 2) * 64
                    srow = rp.tile([1, 512], F32, tag="srow", name="srow")
                    nc.vector.tensor_copy(out=srow, in_=pvt[64:65, :])
                    nc.sync.dma_start(out=r_dram[h, s, :], in_=srow)
                    st4 = rp.tile([4, 128], F32, tag="st4", name="st4")
                    nc.sync.dma_start(
                        out=st4,
                        in_=r_dram[h, s, :].rearrange("(p c) -> p c", p=4))
                    rc4 = rp.tile([4, 128], F32, tag="rc4", name="rc4")
                    nc.vector.reciprocal(out=rc4, in_=st4)
                    nc.sync.dma_start(
                        out=rr_dram[h, s, :].rearrange("(p c) -> p c", p=4),
                        in_=rc4)
                    rb = rp.tile([64, 512], F32, tag="rb", name="rb")
                    rsl = rr_dram[h, s, :]
                    nc.sync.dma_start(
                        out=rb,
                        in_=bass.AP(tensor=rsl.tensor, offset=rsl.offset,
                                    ap=[[0, 64]] + list(rsl.ap)))
                    nc.vector.tensor_mul(
                        out=yts[h // 2][po:po + 64, s * 512:(s + 1) * 512],
                        in0=pvt[0:64, :], in1=rb)

                # ---- main loop: one global slot per (head, key-chunk),
                # scores emitted 4 slots ahead of their PV (so the next
                # head's scores preroll during this head's last slots) ----
                # filler deques; deadlines: V(tb) before PV(h0, tb);
                # K m=0 span sp before scores(h0, 4*sp); K/Q m=1 before
                # scores(h2, *) i.e. by global slot ~28.
                fills = {
                    -1: [lambda: v_chunk(0),
                         lambda: kq_chunk(wk_t, kts, 0, 1),
                         lambda: v_chunk(1),
                         lambda: kq_chunk(wk_t, kts, 0, 2),
                         lambda: v_chunk(2),
                         lambda: v_chunk(3),
                         lambda: v_chunk(4)],
                    0: [lambda: kq_chunk(wk_t, kts, 0, 3)]
                       + [lambda tb=t: v_chunk(tb) for t in range(5, 16)],
                    1: [lambda sp=sp: kq_chunk(wq_t, qts, 1, sp)
                        for sp in range(NS)]
                       + [lambda: kq_chunk(wk_t, kts, 1, 0),
                          lambda: kq_chunk(wk_t, kts, 1, 1)],
                    2: [lambda sp=sp: kq_chunk(wk_t, kts, 1, sp)
                        for sp in (2, 3)],
                    3: [],
                }

                GT = HG * NB
                live = {}
                out_work = []
                pv = None
                for g in range(-7, GT):
                    if g + 7 < GT:
                        live[g + 7] = emit_scores(*divmod(g + 7, NB))
                    if g < 0:
                        if fills[-1]:
                            fills[-1].pop(0)()
                        continue
                    h, ki = divmod(g, NB)
                    if ki == 0:
                        pv = [pvs.tile([128, 512], F32, tag=f"pv{s}",
                                       name=f"pv{s}") for s in range(NS)]
                    if fills[h]:
                        fills[h].pop(0)()
                    elif h == 3 and len(out_work) >= 2:
                        out_pairs([out_work.pop(0), out_work.pop(0)])
                    done = emit_pv(h, ki, live.pop(g), pv)
                    for s in done:
                        finalize(h, s, pv[s])
                        if h == 3:
                            out_work.extend((m, s) for m in range(8))
                # tail: remaining output-projection tiles
                while out_work:
                    out_pairs([out_work.pop(0)
                               for _ in range(min(2, len(out_work)))])
    nc.compile()
    return nc


_NC_CACHE = None


def _get_nc():
    global _NC_CACHE
    if _NC_CACHE is None:
        _NC_CACHE = build_nc()
    return _NC_CACHE


def _w_img(w):
    # SBUF image of a [GW, C] weight slice: wT chunk i ([128, GW]) at
    # columns [i*GW:(i+1)*GW] of the [128, KC*GW] tile
    wT = np.ascontiguousarray(w.T)                     # [C, GW]
    img = wT.reshape(KC, 128, GW).transpose(1, 0, 2).reshape(128, KC * GW)
    return np.ascontiguousarray(img).astype(bfloat16)


def make_in_maps(x, wq, wk, wv, wo):
    x = np.asarray(x, dtype=np.float32)
    wq = np.asarray(wq, dtype=np.float32)
    wk = np.asarray(wk, dtype=np.float32)
    wv = np.asarray(wv, dtype=np.float32)
    wo = np.asarray(wo, dtype=np.float32)
    in_maps = []
    for core in range(N_CORES):
        b, g = core // HG, core % HG
        rows = slice(g * GW, (g + 1) * GW)
        in_maps.append({
            "xT": np.ascontiguousarray(x[b].T).astype(bfloat16),
            "wqT": _w_img(wq[rows, :]),
            "wkT": _w_img(wk[rows, :]),
            "wvT": _w_img(wv[rows, :]),
            "woT": np.ascontiguousarray(wo[:, rows].T).astype(bfloat16),
        })
    return in_maps


def run(x, wq, wk, wv, wo, trace=False, tmpdir=None):
    nc = _get_nc()
    in_maps = make_in_maps(x, wq, wk, wv, wo)
    res = run_bass_kernel_spmd(nc, in_maps, core_ids=list(range(N_CORES)),
                               trace=trace, tmpdir=tmpdir)
    out = np.zeros((B, T, C), dtype=np.float32)
    for core in range(N_CORES):
        out[core // HG] += res.results[core]["outT"].T
    return out, res


def kernel(x, wq, wk, wv, wo):
    out, _ = run(x, wq, wk, wv, wo)
    return out
